# revision 23
# baseline (speedup 1.0000x reference)
"""Multi-head attention (B=1, L=2048, D=1024, H=16) on 8 TRN2 NeuronCores.

Sharding: tensor-parallel over heads. Core i computes heads 2i, 2i+1:
  - projections with column shards of w_q/w_k/w_v (128 cols each)
  - full attention for its 2 heads
  - partial output projection with the matching 128-row shard of w_o
Host sums the 8 partial outputs and adds b_o once.

Fully software-pipelined single pass, built to keep PE dense (HAM-warm) and
ScalarE (exp, the per-core floor: 2*2048^2 elems at 1 elem/cycle/lane)
saturated from ~5us onward:
  - flat loop over 64 iterations: (q-quarter 0..3) x (k-tile 0..15)
  - per iter: PE scores S^T [128,1024] fp32 (2 heads row-packed), ScalarE
    exp -> pt bf16, then (lag 1) PE av (col-packed heads) + denominator
    ones-matmuls accumulated in PSUM (no VectorE reduction work at all)
  - k/v/q projections interleaved into early iterations (PE filler work),
    biases folded into the PSUM->SBUF evacuation on VectorE
  - vh (natural [k,dh] layout) produced by DMA-xbar transposes, not PE
  - per-quarter tail: reciprocal of d, K=1 fp32 broadcast matmul, normalize,
    then output projection + store, all overlapped with the next quarter
  - PSUM budget exactly 8 banks: st 2x2 + av 1 + dc 1 + shared mm 2
"""

import os
import numpy as np
import ml_dtypes

import concourse.bass as bass
import concourse.mybir as mybir
import concourse.tile as tile
from concourse import bacc
from concourse.bass import ts
from concourse.bass_utils import run_bass_kernel_spmd

P = 128
L = 2048
D = 1024
DH = 64
NCORES = 8
NQ = 4  # q quarters
QW = 512  # quarter width
KT = 16  # k tiles of 128
TQ = 8  # contraction chunks of 128 for projections
BF16 = mybir.dt.bfloat16
F32 = mybir.dt.float32
AF = mybir.ActivationFunctionType
ALU = mybir.AluOpType

TRACE = False  # test.py flips this to get an NTFF profile / exec_time_ns
LAST_RESULT = {}

_CACHED_NC = None


def _build():
    nc = bacc.Bacc("TRN2", target_bir_lowering=False, debug=False, num_devices=NCORES)

    qT = nc.dram_tensor("qT", [P, NQ, TQ, QW], BF16, kind="ExternalInput")
    kT = nc.dram_tensor("kT", [P, KT, TQ, P], BF16, kind="ExternalInput")
    vT = nc.dram_tensor("vT", [P, KT, TQ, P], BF16, kind="ExternalInput")
    wq = nc.dram_tensor("wq", [P, TQ, P], BF16, kind="ExternalInput")
    wk = nc.dram_tensor("wk", [P, TQ, P], BF16, kind="ExternalInput")
    wv = nc.dram_tensor("wv", [P, TQ, P], BF16, kind="ExternalInput")
    bq = nc.dram_tensor("bq", [P, 1], F32, kind="ExternalInput")
    bk = nc.dram_tensor("bk", [P, 1], F32, kind="ExternalInput")
    bv = nc.dram_tensor("bv", [P, 1], F32, kind="ExternalInput")
    wo = nc.dram_tensor("wo", [P, D], BF16, kind="ExternalInput")
    out = nc.dram_tensor("out", [KT, P, D], BF16, kind="ExternalOutput")

    with tile.TileContext(nc) as tc:
        with (
            tc.tile_pool(name="const", bufs=1) as const_pool,
            tc.tile_pool(name="inputs", bufs=1) as in_pool,
            tc.tile_pool(name="proj", bufs=1) as proj_pool,
            tc.tile_pool(name="work", bufs=1) as work_pool,
            tc.tile_pool(name="pt_pool", bufs=3) as pt_pool,
            tc.tile_pool(name="osb_pool", bufs=3) as osb_pool,
        ):
            ones_b = const_pool.tile([P, 1], BF16)
            nc.vector.memset(ones_b[:], 1.0)
            ones_f = const_pool.tile([1, DH], F32)
            nc.vector.memset(ones_f[:], 1.0)
            dummy_src = const_pool.tile([P, QW], BF16)
            nc.vector.memset(dummy_src[:], 0.5)
            warm = const_pool.tile([1, 32], F32)
            # preload the exp table set while input DMAs stream
            nc.scalar.activation(warm[:], ones_f[0:1, 0:32], AF.Exp)

            # ---- stage inputs ----
            wq_sb = in_pool.tile([P, TQ, P], BF16)
            wk_sb = in_pool.tile([P, TQ, P], BF16)
            wv_sb = in_pool.tile([P, TQ, P], BF16)
            bq_sb = in_pool.tile([P, 1], F32)
            bk_sb = in_pool.tile([P, 1], F32)
            bv_sb = in_pool.tile([P, 1], F32)
            wo_sb = in_pool.tile([P, D], BF16)
            qT_sb = in_pool.tile([P, NQ, TQ, QW], BF16)
            kT_sb = in_pool.tile([P, KT, TQ, P], BF16)
            vT_sb = in_pool.tile([P, KT, TQ, P], BF16)

            # bulk input on the two HWDGE rings: sync = k path, scalar = q0+v
            # path (scalar's queue is idle until the first exp anyway; these
            # triggers all precede the exps in its FIFO). gpsimd/SWDGE is an
            # order of magnitude slower — only late-needed q quarters ride it.
            nc.sync.dma_start(wq_sb[:], wq[:])
            nc.sync.dma_start(bq_sb[:], bq[:])
            nc.scalar.dma_start(wk_sb[:], wk[:])
            nc.scalar.dma_start(bk_sb[:], bk[:])
            nc.sync.dma_start(kT_sb[:, 0:2], kT[:, 0:2])
            nc.scalar.dma_start(qT_sb[:, 0], qT[:, 0])
            nc.gpsimd.dma_start(qT_sb[:, 1], qT[:, 1])
            nc.gpsimd.dma_start(qT_sb[:, 2], qT[:, 2])
            nc.gpsimd.dma_start(qT_sb[:, 3], qT[:, 3])
            nc.scalar.dma_start(wv_sb[:], wv[:])
            nc.scalar.dma_start(bv_sb[:], bv[:])
            nc.sync.dma_start(kT_sb[:, 2:4], kT[:, 2:4])
            nc.scalar.dma_start(vT_sb[:, 0:2], vT[:, 0:2])
            nc.sync.dma_start(kT_sb[:, 4:8], kT[:, 4:8])
            nc.scalar.dma_start(vT_sb[:, 2:4], vT[:, 2:4])
            nc.sync.dma_start(kT_sb[:, 8:12], kT[:, 8:12])
            nc.scalar.dma_start(vT_sb[:, 4:8], vT[:, 4:8])
            nc.sync.dma_start(kT_sb[:, 12:16], kT[:, 12:16])
            nc.scalar.dma_start(vT_sb[:, 8:12], vT[:, 8:12])
            nc.scalar.dma_start(vT_sb[:, 12:16], vT[:, 12:16])
            nc.scalar.dma_start(wo_sb[:], wo[:])

            # projection outputs (heads on partitions: h*64..h*64+63)
            qhT = proj_pool.tile([P, L], BF16)
            khT = proj_pool.tile([P, L], BF16)
            vhT = proj_pool.tile([P, L], BF16)
            vh = proj_pool.tile([P, KT, P], BF16)  # natural [k, dh] layout

            lhsT_c = work_pool.tile([P, L], BF16)  # normalized concat^T
            u_sb = work_pool.tile([P, QW], F32)
            dsb = work_pool.tile([1, 2 * QW], F32)  # d rows gathered
            dsp = work_pool.tile([DH, 2 * QW // DH], F32)  # spread for recip
            dcr = work_pool.tile([1, 2 * QW], F32)  # 1/d back in row layout

            with (
                tc.tile_pool(name="st_ps", bufs=2, space="PSUM") as st_ps,
                tc.tile_pool(name="av_ps", bufs=1, space="PSUM") as av_ps,
                tc.tile_pool(name="dc_ps", bufs=1, space="PSUM") as dc_ps,
                tc.tile_pool(name="mm_ps", bufs=2, space="PSUM") as mm_ps,
            ):

                def qproj(qi):
                    ps = mm_ps.tile([P, QW], F32, tag="mm", name=f"qp{qi}")
                    for t in range(TQ):
                        nc.tensor.matmul(
                            ps[:],
                            wq_sb[:, t, :],
                            qT_sb[:, qi, t, :],
                            start=(t == 0),
                            stop=(t == TQ - 1),
                        )
                    nc.vector.tensor_scalar(
                        qhT[:, ts(qi, QW)], ps[:], bq_sb[:], None, op0=ALU.add
                    )

                def kproj(g):
                    ps = mm_ps.tile([P, QW], F32, tag="mm", name=f"kp{g}")
                    for t in range(TQ):
                        nc.tensor.matmul(
                            ps[:],
                            wk_sb[:, t, :],
                            kT_sb[:, ts(g, 4), t, :],
                            start=(t == 0),
                            stop=(t == TQ - 1),
                        )
                    nc.vector.tensor_scalar(
                        khT[:, ts(g, QW)], ps[:], bk_sb[:], None, op0=ALU.add
                    )

                def vproj(g):
                    ps = mm_ps.tile([P, QW], F32, tag="mm", name=f"vp{g}")
                    for t in range(TQ):
                        nc.tensor.matmul(
                            ps[:],
                            wv_sb[:, t, :],
                            vT_sb[:, ts(g, 4), t, :],
                            start=(t == 0),
                            stop=(t == TQ - 1),
                        )
                    nc.vector.tensor_scalar(
                        vhT[:, ts(g, QW)], ps[:], bv_sb[:], None, op0=ALU.add
                    )
                    # natural layout via DMA xbar transpose: [dh, k] -> [k, dh]
                    nc.sync.dma_start_transpose(
                        vh[:, ts(g, 4), :], vhT[:, ts(g, QW)]
                    )

                # ---- PE pre-warm: ~5us of throwaway matmuls while the input
                # DMAs stream, so HAM un-throttles before the real work ----
                warmps = mm_ps.tile([P, QW], F32, tag="mm", name="warmps")
                for _ in range(16):
                    nc.tensor.matmul(warmps[:], dummy_src[:, 0:P], dummy_src[:])

                # ---- head: first quads so the pipeline can start ----
                kproj(0)
                qproj(0)

                st_tiles = {}
                pt_tiles = {}
                av_t = None
                dc_t = None

                def do_st_exp(it):
                    qi, kt = divmod(it, KT)
                    st_t = st_ps.tile([P, 2 * QW], F32, tag="st", name=f"st{it}")
                    for h in (0, 1):
                        nc.tensor.matmul(
                            st_t[:, ts(h, QW)],
                            khT[ts(h, DH), ts(kt, P)],
                            qhT[ts(h, DH), ts(qi, QW)],
                        )
                    pt_t = pt_pool.tile([P, 2 * QW], BF16, tag="pt", name=f"pt{it}")
                    nc.scalar.activation(pt_t[:], st_t[:], AF.Exp, scale=0.125)
                    st_tiles[it] = st_t
                    pt_tiles[it] = pt_t

                def do_av_dc(it):
                    nonlocal av_t, dc_t
                    qi, kt = divmod(it, KT)
                    if kt == 0:
                        av_t = av_ps.tile([P, QW], F32, tag="av", name=f"av{qi}")
                        dc_t = dc_ps.tile([33, QW], F32, tag="dc", name=f"dc{qi}")
                    pt_t = pt_tiles.pop(it)
                    first = kt == 0
                    last = kt == KT - 1
                    for h in (0, 1):
                        nc.tensor.matmul(
                            av_t[ts(h, DH), :],
                            vh[:, kt, ts(h, DH)],
                            pt_t[:, ts(h, QW)],
                            start=first,
                            stop=last,
                            tile_position=(0, DH * h),
                        )
                    nc.tensor.matmul(
                        dc_t[0:1, :],
                        ones_b[:, 0:1],
                        pt_t[:, 0:QW],
                        start=first,
                        stop=last,
                        tile_position=(0, 0),
                    )
                    nc.tensor.matmul(
                        dc_t[32:33, :],
                        ones_b[:, 0:1],
                        pt_t[:, QW : 2 * QW],
                        start=first,
                        stop=last,
                        tile_position=(0, 32),
                    )
                    st_tiles.pop(it)

                def quarter_tail(qi, last=False):
                    # d -> 1/d (reciprocal on a partition-spread copy: DVE
                    # reciprocal cost is ~6.5ns/elem of free dim) -> broadcast
                    # over head partition groups -> normalize
                    nc.vector.tensor_copy(u_sb[:], av_t[:])
                    nc.vector.tensor_copy(dsb[0:1, 0:QW], dc_t[0:1, :])
                    nc.vector.tensor_copy(dsb[0:1, QW : 2 * QW], dc_t[32:33, :])
                    nc.sync.dma_start(dsp[:], dsb[:])
                    nc.vector.reciprocal(dsp[:], dsp[:])
                    nc.sync.dma_start(dcr[:], dsp[:])
                    bc_t = mm_ps.tile([P, QW], F32, tag="mm", name=f"bc{qi}")
                    nc.tensor.matmul(
                        bc_t[0:DH, :],
                        ones_f[0:1, :],
                        dcr[0:1, 0:QW],
                        tile_position=(0, 0),
                    )
                    nc.tensor.matmul(
                        bc_t[DH:P, :],
                        ones_f[0:1, :],
                        dcr[0:1, QW : 2 * QW],
                        tile_position=(0, DH),
                    )
                    if last:
                        # slice the normalize per m-tile so outproj can chase it
                        for j in range(4):
                            nc.vector.tensor_tensor(
                                lhsT_c[:, qi * QW + j * P : qi * QW + (j + 1) * P],
                                u_sb[:, ts(j, P)],
                                bc_t[:, ts(j, P)],
                                op=ALU.mult,
                            )
                    else:
                        nc.vector.tensor_tensor(
                            lhsT_c[:, ts(qi, QW)], u_sb[:], bc_t[:], op=ALU.mult
                        )

                def outproj(m, on_scalar=False):
                    osb = osb_pool.tile([P, D], BF16, tag="osb", name=f"osb{m}")
                    for n in (0, 1):
                        ps = mm_ps.tile([P, QW], F32, tag="mm", name=f"op{m}_{n}")
                        nc.tensor.matmul(
                            ps[:], lhsT_c[:, ts(m, P)], wo_sb[:, ts(n, QW)]
                        )
                        if on_scalar:
                            nc.scalar.copy(osb[:, ts(n, QW)], ps[:])
                        else:
                            nc.vector.tensor_copy(osb[:, ts(n, QW)], ps[:])
                    nc.sync.dma_start(out[m], osb[:])

                # ---- main pipelined loop ----
                for it in range(64 + 1):
                    qi, kt = divmod(it, KT)
                    if it < 64:
                        do_st_exp(it)
                    # projection filler, scheduled just-in-time for quarter 0
                    if it == 0:
                        vproj(0)
                    elif it == 2:
                        kproj(1)
                    elif it == 4:
                        vproj(1)
                    elif it == 6:
                        kproj(2)
                    elif it == 8:
                        vproj(2)
                    elif it == 10:
                        kproj(3)
                    elif it == 12:
                        vproj(3)
                    elif kt == 13 and qi < 3:
                        qproj(qi + 1)
                    if it > 0:
                        do_av_dc(it - 1)
                    if it >= KT and kt == 0:
                        quarter_tail(qi - 1, last=(qi - 1 == 3))
                    # spread output projection of the previous quarter
                    if it > KT and kt in (4, 6, 8, 10):
                        outproj((qi - 1) * 4 + (kt - 4) // 2)
                # last quarter's output projection (tail ran at it=64);
                # ScalarE is idle after the final exp — use it for the evacs
                for m in range(12, 16):
                    outproj(m, on_scalar=True)

    nc.compile()
    return nc


def kernel(q, k, v, w_q, b_q, w_k, b_k, w_v, b_v, w_o, b_o):
    global _CACHED_NC, LAST_RESULT
    if _CACHED_NC is None:
        _CACHED_NC = _build()
    nc = _CACHED_NC

    bf16 = ml_dtypes.bfloat16

    qTf = np.ascontiguousarray(np.asarray(q, np.float32)[0].T)  # [D, L]
    kTf = np.ascontiguousarray(np.asarray(k, np.float32)[0].T)
    vTf = np.ascontiguousarray(np.asarray(v, np.float32)[0].T)
    q2 = np.ascontiguousarray(
        qTf.reshape(TQ, P, NQ, QW).transpose(1, 2, 0, 3)
    ).astype(bf16)
    k2 = np.ascontiguousarray(
        kTf.reshape(TQ, P, KT, P).transpose(1, 2, 0, 3)
    ).astype(bf16)
    v2 = np.ascontiguousarray(
        vTf.reshape(TQ, P, KT, P).transpose(1, 2, 0, 3)
    ).astype(bf16)

    w_q = np.asarray(w_q, np.float32)
    w_k = np.asarray(w_k, np.float32)
    w_v = np.asarray(w_v, np.float32)
    w_o = np.asarray(w_o, np.float32)
    b_q = np.asarray(b_q, np.float32)
    b_k = np.asarray(b_k, np.float32)
    b_v = np.asarray(b_v, np.float32)
    b_o = np.asarray(b_o, np.float32)

    def tile_w(w):  # [D, 128] -> [128, D//128, 128] contiguous
        return np.ascontiguousarray(
            w.reshape(TQ, P, P).transpose(1, 0, 2)
        ).astype(bf16)

    in_maps = []
    for i in range(NCORES):
        sl = slice(P * i, P * (i + 1))
        in_maps.append(
            {
                "qT": q2,
                "kT": k2,
                "vT": v2,
                "wq": tile_w(w_q[:, sl]),
                "wk": tile_w(w_k[:, sl]),
                "wv": tile_w(w_v[:, sl]),
                "bq": np.ascontiguousarray(b_q[sl]).reshape(P, 1),
                "bk": np.ascontiguousarray(b_k[sl]).reshape(P, 1),
                "bv": np.ascontiguousarray(b_v[sl]).reshape(P, 1),
                "wo": np.ascontiguousarray(w_o[sl, :]).astype(bf16),
            }
        )

    kwargs = {}
    if TRACE:
        import shutil

        tdir = "/tmp/bass_trace"
        shutil.rmtree(tdir, ignore_errors=True)
        os.makedirs(tdir, exist_ok=True)
        kwargs["tmpdir"] = tdir
    res = run_bass_kernel_spmd(nc, in_maps, list(range(NCORES)), trace=TRACE, **kwargs)
    LAST_RESULT = {
        "exec_time_ns": res.exec_time_ns,
        "trace_path": (res.instructions_and_trace or (None, None))[1],
    }
    acc = np.zeros((L, D), np.float64)
    for i in range(NCORES):
        acc += res.results[i]["out"].reshape(L, D).astype(np.float64)
    acc += b_o.astype(np.float64)
    return acc.astype(np.float32).reshape(1, L, D)


# revision 28
# speedup vs baseline: 1.0766x; 1.0766x over previous
"""Multi-head attention (B=1, L=2048, D=1024, H=16) on 8 TRN2 NeuronCores.

Sharding: tensor-parallel over heads. Core i computes heads 2i, 2i+1:
  - projections with column shards of w_q/w_k/w_v (128 cols each)
  - full attention for its 2 heads
  - partial output projection with the matching 128-row shard of w_o
Host sums the 8 partial outputs and adds b_o once.

Fully software-pipelined single pass, built to keep PE dense (HAM-warm) and
ScalarE (exp, the per-core floor: 2*2048^2 elems at 1 elem/cycle/lane)
saturated from ~5us onward:
  - flat loop over 64 iterations: (q-quarter 0..3) x (k-tile 0..15)
  - per iter: PE scores S^T [128,1024] fp32 (2 heads row-packed), ScalarE
    exp -> pt bf16, then (lag 1) PE av (col-packed heads) + denominator
    ones-matmuls accumulated in PSUM (no VectorE reduction work at all)
  - k/v/q projections interleaved into early iterations (PE filler work),
    biases folded into the PSUM->SBUF evacuation on VectorE
  - vh (natural [k,dh] layout) produced by DMA-xbar transposes, not PE
  - per-quarter tail: reciprocal of d, K=1 fp32 broadcast matmul, normalize,
    then output projection + store, all overlapped with the next quarter
  - PSUM budget exactly 8 banks: st 2x2 + av 1 + dc 1 + shared mm 2
"""

import os
import numpy as np
import ml_dtypes

import concourse.bass as bass
import concourse.mybir as mybir
import concourse.tile as tile
from concourse import bacc
from concourse.bass import ts
from concourse.bass_utils import run_bass_kernel_spmd
from concourse.masks import make_identity

P = 128
L = 2048
D = 1024
DH = 64
NCORES = 8
NQ = 4  # q quarters
QW = 512  # quarter width
KT = 16  # k tiles of 128
TQ = 8  # contraction chunks of 128 for projections
BF16 = mybir.dt.bfloat16
F32 = mybir.dt.float32
AF = mybir.ActivationFunctionType
ALU = mybir.AluOpType

TRACE = False  # test.py flips this to get an NTFF profile / exec_time_ns
LAST_RESULT = {}

_CACHED_NC = None


def _build():
    nc = bacc.Bacc("TRN2", target_bir_lowering=False, debug=False, num_devices=NCORES)

    qT = nc.dram_tensor("qT", [P, NQ, TQ, QW], BF16, kind="ExternalInput")
    kT = nc.dram_tensor("kT", [P, KT, TQ, P], BF16, kind="ExternalInput")
    vT = nc.dram_tensor("vT", [P, KT, TQ, P], BF16, kind="ExternalInput")
    wq = nc.dram_tensor("wq", [P, TQ, P], BF16, kind="ExternalInput")
    wk = nc.dram_tensor("wk", [P, TQ, P], BF16, kind="ExternalInput")
    wv = nc.dram_tensor("wv", [P, TQ, P], BF16, kind="ExternalInput")
    bq = nc.dram_tensor("bq", [P, 1], F32, kind="ExternalInput")
    bk = nc.dram_tensor("bk", [P, 1], F32, kind="ExternalInput")
    bv = nc.dram_tensor("bv", [P, 1], F32, kind="ExternalInput")
    wo = nc.dram_tensor("wo", [P, D], BF16, kind="ExternalInput")
    out = nc.dram_tensor("out", [KT, P, D], BF16, kind="ExternalOutput")

    with tile.TileContext(nc) as tc:
        with (
            tc.tile_pool(name="const", bufs=1) as const_pool,
            tc.tile_pool(name="inputs", bufs=1) as in_pool,
            tc.tile_pool(name="proj", bufs=1) as proj_pool,
            tc.tile_pool(name="work", bufs=1) as work_pool,
            tc.tile_pool(name="pt_pool", bufs=3) as pt_pool,
            tc.tile_pool(name="osb_pool", bufs=3) as osb_pool,
        ):
            ones_b = const_pool.tile([P, 1], BF16)
            nc.vector.memset(ones_b[:], 1.0)
            ones_f = const_pool.tile([1, DH], F32)
            nc.vector.memset(ones_f[:], 1.0)
            dummy_src = const_pool.tile([P, QW], BF16)
            nc.vector.memset(dummy_src[:], 0.5)
            identity = const_pool.tile([P, P], BF16)
            make_identity(nc, identity[:])
            warm = const_pool.tile([1, 32], F32)
            # preload the exp table set while input DMAs stream
            nc.scalar.activation(warm[:], ones_f[0:1, 0:32], AF.Exp)

            # ---- stage inputs ----
            wq_sb = in_pool.tile([P, TQ, P], BF16)
            wk_sb = in_pool.tile([P, TQ, P], BF16)
            wv_sb = in_pool.tile([P, TQ, P], BF16)
            bq_sb = in_pool.tile([P, 1], F32)
            bk_sb = in_pool.tile([P, 1], F32)
            bv_sb = in_pool.tile([P, 1], F32)
            wo_sb = in_pool.tile([P, D], BF16)
            qT_sb = in_pool.tile([P, NQ, TQ, QW], BF16)
            kT_sb = in_pool.tile([P, KT, TQ, P], BF16)
            vT_sb = in_pool.tile([P, KT, TQ, P], BF16)

            # All bulk input on the sync HWDGE ring in consumption order.
            # Triggers block in-queue on ring depth, so the scalar queue must
            # stay clear of bulk DMA or the exps starve behind them; tiny
            # weights + late-needed q quarters ride the slow gpsimd/SWDGE.
            nc.gpsimd.dma_start(wk_sb[:], wk[:])
            nc.gpsimd.dma_start(bk_sb[:], bk[:])
            nc.gpsimd.dma_start(wv_sb[:], wv[:])
            nc.gpsimd.dma_start(bv_sb[:], bv[:])
            nc.sync.dma_start(wq_sb[:], wq[:])
            nc.sync.dma_start(bq_sb[:], bq[:])
            nc.sync.dma_start(kT_sb[:, 0:2], kT[:, 0:2])
            nc.sync.dma_start(qT_sb[:, 0], qT[:, 0])
            nc.sync.dma_start(kT_sb[:, 2:4], kT[:, 2:4])
            nc.sync.dma_start(vT_sb[:, 0:2], vT[:, 0:2])
            nc.sync.dma_start(vT_sb[:, 2:4], vT[:, 2:4])
            nc.sync.dma_start(kT_sb[:, 4:8], kT[:, 4:8])
            nc.sync.dma_start(vT_sb[:, 4:8], vT[:, 4:8])
            nc.sync.dma_start(kT_sb[:, 8:12], kT[:, 8:12])
            nc.sync.dma_start(vT_sb[:, 8:12], vT[:, 8:12])
            nc.sync.dma_start(kT_sb[:, 12:16], kT[:, 12:16])
            nc.sync.dma_start(vT_sb[:, 12:16], vT[:, 12:16])
            nc.sync.dma_start(qT_sb[:, 1], qT[:, 1])
            nc.gpsimd.dma_start(qT_sb[:, 2], qT[:, 2])
            nc.gpsimd.dma_start(qT_sb[:, 3], qT[:, 3])
            nc.gpsimd.dma_start(wo_sb[:], wo[:])

            # projection outputs (heads on partitions: h*64..h*64+63)
            qhT = proj_pool.tile([P, L], BF16)
            khT = proj_pool.tile([P, L], BF16)
            vhT = proj_pool.tile([P, L], BF16)
            vh = proj_pool.tile([P, KT, P], BF16)  # natural [k, dh] layout

            lhsT_c = work_pool.tile([P, L], BF16)  # normalized concat^T
            u_sb = work_pool.tile([P, QW], F32)
            dsb = work_pool.tile([1, 2 * QW], F32)  # d rows gathered
            dsp = work_pool.tile([DH, 2 * QW // DH], F32)  # spread for recip
            dcr = work_pool.tile([1, 2 * QW], F32)  # 1/d back in row layout

            with (
                tc.tile_pool(name="st_ps", bufs=2, space="PSUM") as st_ps,
                tc.tile_pool(name="av_ps", bufs=1, space="PSUM") as av_ps,
                tc.tile_pool(name="dc_ps", bufs=1, space="PSUM") as dc_ps,
                tc.tile_pool(name="mm_ps", bufs=2, space="PSUM") as mm_ps,
            ):

                def qproj(qi):
                    ps = mm_ps.tile([P, QW], F32, tag="mm", name=f"qp{qi}")
                    for t in range(TQ):
                        nc.tensor.matmul(
                            ps[:],
                            wq_sb[:, t, :],
                            qT_sb[:, qi, t, :],
                            start=(t == 0),
                            stop=(t == TQ - 1),
                        )
                    nc.vector.tensor_scalar(
                        qhT[:, ts(qi, QW)], ps[:], bq_sb[:], None, op0=ALU.add
                    )

                def kproj(g):
                    ps = mm_ps.tile([P, QW], F32, tag="mm", name=f"kp{g}")
                    for t in range(TQ):
                        nc.tensor.matmul(
                            ps[:],
                            wk_sb[:, t, :],
                            kT_sb[:, ts(g, 4), t, :],
                            start=(t == 0),
                            stop=(t == TQ - 1),
                        )
                    nc.vector.tensor_scalar(
                        khT[:, ts(g, QW)], ps[:], bk_sb[:], None, op0=ALU.add
                    )

                def vproj(g):
                    ps = mm_ps.tile([P, QW], F32, tag="mm", name=f"vp{g}")
                    for t in range(TQ):
                        nc.tensor.matmul(
                            ps[:],
                            wv_sb[:, t, :],
                            vT_sb[:, ts(g, 4), t, :],
                            start=(t == 0),
                            stop=(t == TQ - 1),
                        )
                    nc.vector.tensor_scalar(
                        vhT[:, ts(g, QW)], ps[:], bv_sb[:], None, op0=ALU.add
                    )
                    # natural layout via PE transpose (off the DMA rings: the
                    # sync ring is clogged with input triggers early on)
                    for j in range(4):
                        kt = 4 * g + j
                        pst = mm_ps.tile(
                            [P, P], BF16, tag="mm", name=f"vt{kt}"
                        )
                        nc.tensor.transpose(
                            pst[:], vhT[:, ts(kt, P)], identity[:]
                        )
                        nc.vector.tensor_copy(vh[:, kt, :], pst[:])

                # ---- PE pre-warm: ~5us of throwaway matmuls while the input
                # DMAs stream, so HAM un-throttles before the real work ----
                warmps = mm_ps.tile([P, QW], F32, tag="mm", name="warmps")
                for _ in range(16):
                    nc.tensor.matmul(warmps[:], dummy_src[:, 0:P], dummy_src[:])

                # ---- head: first quads so the pipeline can start ----
                kproj(0)
                qproj(0)

                st_tiles = {}
                pt_tiles = {}
                av_t = None
                dc_t = None

                def do_st_exp(it):
                    qi, kt = divmod(it, KT)
                    st_t = st_ps.tile([P, 2 * QW], F32, tag="st", name=f"st{it}")
                    for h in (0, 1):
                        nc.tensor.matmul(
                            st_t[:, ts(h, QW)],
                            khT[ts(h, DH), ts(kt, P)],
                            qhT[ts(h, DH), ts(qi, QW)],
                        )
                    pt_t = pt_pool.tile([P, 2 * QW], BF16, tag="pt", name=f"pt{it}")
                    nc.scalar.activation(pt_t[:], st_t[:], AF.Exp, scale=0.125)
                    st_tiles[it] = st_t
                    pt_tiles[it] = pt_t

                def do_av_dc(it):
                    nonlocal av_t, dc_t
                    qi, kt = divmod(it, KT)
                    if kt == 0:
                        av_t = av_ps.tile([P, QW], F32, tag="av", name=f"av{qi}")
                        dc_t = dc_ps.tile([33, QW], F32, tag="dc", name=f"dc{qi}")
                    pt_t = pt_tiles.pop(it)
                    first = kt == 0
                    last = kt == KT - 1
                    for h in (0, 1):
                        nc.tensor.matmul(
                            av_t[ts(h, DH), :],
                            vh[:, kt, ts(h, DH)],
                            pt_t[:, ts(h, QW)],
                            start=first,
                            stop=last,
                            tile_position=(0, DH * h),
                        )
                    nc.tensor.matmul(
                        dc_t[0:1, :],
                        ones_b[:, 0:1],
                        pt_t[:, 0:QW],
                        start=first,
                        stop=last,
                        tile_position=(0, 0),
                    )
                    nc.tensor.matmul(
                        dc_t[32:33, :],
                        ones_b[:, 0:1],
                        pt_t[:, QW : 2 * QW],
                        start=first,
                        stop=last,
                        tile_position=(0, 32),
                    )
                    st_tiles.pop(it)

                def quarter_tail(qi, last=False):
                    # d -> 1/d (reciprocal on a partition-spread copy: DVE
                    # reciprocal cost is ~6.5ns/elem of free dim) -> broadcast
                    # over head partition groups -> normalize
                    nc.vector.tensor_copy(u_sb[:], av_t[:])
                    nc.vector.tensor_copy(dsb[0:1, 0:QW], dc_t[0:1, :])
                    nc.vector.tensor_copy(dsb[0:1, QW : 2 * QW], dc_t[32:33, :])
                    nc.gpsimd.dma_start(dsp[:], dsb[:])
                    nc.vector.reciprocal(dsp[:], dsp[:])
                    nc.gpsimd.dma_start(dcr[:], dsp[:])
                    bc_t = mm_ps.tile([P, QW], F32, tag="mm", name=f"bc{qi}")
                    nc.tensor.matmul(
                        bc_t[0:DH, :],
                        ones_f[0:1, :],
                        dcr[0:1, 0:QW],
                        tile_position=(0, 0),
                    )
                    nc.tensor.matmul(
                        bc_t[DH:P, :],
                        ones_f[0:1, :],
                        dcr[0:1, QW : 2 * QW],
                        tile_position=(0, DH),
                    )
                    if last:
                        # slice the normalize per m-tile so outproj can chase it
                        for j in range(4):
                            nc.vector.tensor_tensor(
                                lhsT_c[:, qi * QW + j * P : qi * QW + (j + 1) * P],
                                u_sb[:, ts(j, P)],
                                bc_t[:, ts(j, P)],
                                op=ALU.mult,
                            )
                    else:
                        nc.vector.tensor_tensor(
                            lhsT_c[:, ts(qi, QW)], u_sb[:], bc_t[:], op=ALU.mult
                        )

                def outproj(m, on_scalar=False):
                    osb = osb_pool.tile([P, D], BF16, tag="osb", name=f"osb{m}")
                    for n in (0, 1):
                        ps = mm_ps.tile([P, QW], F32, tag="mm", name=f"op{m}_{n}")
                        nc.tensor.matmul(
                            ps[:], lhsT_c[:, ts(m, P)], wo_sb[:, ts(n, QW)]
                        )
                        if on_scalar:
                            nc.scalar.copy(osb[:, ts(n, QW)], ps[:])
                        else:
                            nc.vector.tensor_copy(osb[:, ts(n, QW)], ps[:])
                    nc.sync.dma_start(out[m], osb[:])

                # ---- main pipelined loop ----
                for it in range(64 + 1):
                    qi, kt = divmod(it, KT)
                    if it < 64:
                        do_st_exp(it)
                    # projection filler, scheduled just-in-time for quarter 0
                    if it == 0:
                        vproj(0)
                    elif it == 2:
                        kproj(1)
                    elif it == 4:
                        vproj(1)
                    elif it == 6:
                        kproj(2)
                    elif it == 8:
                        vproj(2)
                    elif it == 10:
                        kproj(3)
                    elif it == 12:
                        vproj(3)
                    elif kt == 13 and qi < 3:
                        qproj(qi + 1)
                    if it > 0:
                        do_av_dc(it - 1)
                    if it >= KT and kt == 0:
                        quarter_tail(qi - 1, last=(qi - 1 == 3))
                    # spread output projection of the previous quarter
                    if it > KT and kt in (4, 6, 8, 10):
                        outproj((qi - 1) * 4 + (kt - 4) // 2)
                # last quarter's output projection (tail ran at it=64);
                # ScalarE is idle after the final exp — use it for the evacs
                for m in range(12, 16):
                    outproj(m, on_scalar=True)

    nc.compile()
    return nc


def kernel(q, k, v, w_q, b_q, w_k, b_k, w_v, b_v, w_o, b_o):
    global _CACHED_NC, LAST_RESULT
    if _CACHED_NC is None:
        _CACHED_NC = _build()
    nc = _CACHED_NC

    bf16 = ml_dtypes.bfloat16

    qTf = np.ascontiguousarray(np.asarray(q, np.float32)[0].T)  # [D, L]
    kTf = np.ascontiguousarray(np.asarray(k, np.float32)[0].T)
    vTf = np.ascontiguousarray(np.asarray(v, np.float32)[0].T)
    q2 = np.ascontiguousarray(
        qTf.reshape(TQ, P, NQ, QW).transpose(1, 2, 0, 3)
    ).astype(bf16)
    k2 = np.ascontiguousarray(
        kTf.reshape(TQ, P, KT, P).transpose(1, 2, 0, 3)
    ).astype(bf16)
    v2 = np.ascontiguousarray(
        vTf.reshape(TQ, P, KT, P).transpose(1, 2, 0, 3)
    ).astype(bf16)

    w_q = np.asarray(w_q, np.float32)
    w_k = np.asarray(w_k, np.float32)
    w_v = np.asarray(w_v, np.float32)
    w_o = np.asarray(w_o, np.float32)
    b_q = np.asarray(b_q, np.float32)
    b_k = np.asarray(b_k, np.float32)
    b_v = np.asarray(b_v, np.float32)
    b_o = np.asarray(b_o, np.float32)

    def tile_w(w):  # [D, 128] -> [128, D//128, 128] contiguous
        return np.ascontiguousarray(
            w.reshape(TQ, P, P).transpose(1, 0, 2)
        ).astype(bf16)

    in_maps = []
    for i in range(NCORES):
        sl = slice(P * i, P * (i + 1))
        in_maps.append(
            {
                "qT": q2,
                "kT": k2,
                "vT": v2,
                "wq": tile_w(w_q[:, sl]),
                "wk": tile_w(w_k[:, sl]),
                "wv": tile_w(w_v[:, sl]),
                "bq": np.ascontiguousarray(b_q[sl]).reshape(P, 1),
                "bk": np.ascontiguousarray(b_k[sl]).reshape(P, 1),
                "bv": np.ascontiguousarray(b_v[sl]).reshape(P, 1),
                "wo": np.ascontiguousarray(w_o[sl, :]).astype(bf16),
            }
        )

    kwargs = {}
    if TRACE:
        import shutil

        tdir = "/tmp/bass_trace"
        shutil.rmtree(tdir, ignore_errors=True)
        os.makedirs(tdir, exist_ok=True)
        kwargs["tmpdir"] = tdir
    res = run_bass_kernel_spmd(nc, in_maps, list(range(NCORES)), trace=TRACE, **kwargs)
    LAST_RESULT = {
        "exec_time_ns": res.exec_time_ns,
        "trace_path": (res.instructions_and_trace or (None, None))[1],
    }
    acc = np.zeros((L, D), np.float64)
    for i in range(NCORES):
        acc += res.results[i]["out"].reshape(L, D).astype(np.float64)
    acc += b_o.astype(np.float64)
    return acc.astype(np.float32).reshape(1, L, D)
